# revision 1
# baseline (speedup 1.0000x reference)
"""Trainium2 Bass kernel for nn_DetectionLoss (YOLO-style detection loss).

Strategy (data-parallel over batch, 2 images per core x 8 cores; default v3):
  Dense part: obj BCE at non-positive cells reduces to sum(softplus(pred[...,4]))
    over the grid, computed from a host-packed planar channel-4 tile (200KB/core
    instead of streaming the full 17MB of pred rows).
  Sparse part: the 96 gt entries per core fetch their 9 anchor rows (3 scales x
    3 anchors) with ONE multi-index dma_gather (int16 index block built on
    device via a PE one-hot matmul). Anchor IoU / per-cell dedup / decode /
    CIoU / BCE run as x/y-paired [96,18] vector ops; cross-entry same-cell
    maxima use DVE 32x32 transposes + PE outer-product broadcasts into PSUM
    (no DRAM round trip). Arctan runs on the Scalar engine (range-reduced to
    [0,1]); exp/ln and sigmoid/arctan are pinned to two activation table sets
    with exactly two table loads.
  Each core ships raw per-entry partial sums ([96,45] + dense [128,3]); the
    host sums them (the "all-reduce" of the sharding hint) and normalizes.
  v1/v2 (earlier, slower variants) are kept for A/B via kernel(_variant=...).
"""

import numpy as np

import concourse.bacc as bacc
import concourse.bass as bass
import concourse.tile as tile
from concourse import mybir
from concourse.bass_utils import run_bass_kernel_spmd

F32 = mybir.dt.float32
I32 = mybir.dt.int32
AF = mybir.ActivationFunctionType
OP = mybir.AluOpType
AX = mybir.AxisListType

# ---- problem constants (hardcoded per contract) ----
B, N, A, C = 16, 48, 3, 80
NCORES = 8
BLOC = B // NCORES          # 2 images per core
NP = BLOC * N               # 96 entry partitions
STRIDES = (8.0, 16.0, 32.0)
WS = (80, 40, 20)
HWS = (6400, 1600, 400)
RS = [BLOC * A * hw for hw in HWS]            # rows per scale per core
SBASE = [0, RS[0], RS[0] + RS[1]]             # scale row bases
ROWS = RS[0] + RS[1] + RS[2]                  # 50400
ROWS_PAD = 50688                              # 396 * 128, divisible by 3
NCOL = [300, 75, 21]                          # rows per partition per scale (s2 padded)
PAD_VAL = -60.0                               # softplus(PAD_VAL) == 0 in f32
EPS = 1e-7
# dense streaming chunks: (scale, col_start, width) in rows-per-partition units
CHUNKS = [(0, 0, 75), (0, 75, 75), (0, 150, 75), (0, 225, 75), (1, 0, 75), (2, 0, 21)]

# atan(z)/z poly in z^2 on [0,1], max abs err 5.8e-7
ATAN_C = [0.9999997152904466, -0.33327976036522494, 0.1989502583419013,
          -0.1353767514232845, 0.08475969773639125, -0.03775170756922951,
          0.008097294930236264]

_CACHE = {}
NUM_SWDGE_QUEUES = 1

# Pin exp/ln activations to the one table set containing both, so the
# compiler emits a single ACT_TABLE_LOAD instead of thrashing (~2.7us each).
# Positions in the list are preserved (they are the act_func_set ids).
_orig_get_act_tables = bacc.get_activation_tables


def _pinned_act_tables(arch):
    tables = _orig_get_act_tables(arch)
    keep = "natural_log_exp_and_others"
    if keep in tables:
        for name, funcs in tables.items():
            if name != keep:
                funcs.discard(AF.Exp)
                funcs.discard(AF.Ln)
    keep2 = "sigmoid_and_others"
    if keep2 in tables:
        for name, funcs in tables.items():
            if name != keep2:
                funcs.discard(AF.Arctan)
                funcs.discard(AF.Sigmoid)
    return tables


bacc.get_activation_tables = _pinned_act_tables

BATCH_GATHER = False
GATHER_OFF = False
GATHER_SPLIT = True
FULL_DEDUP = False
DEBUG_G = False


def _rap(ap, off_elems, pattern):
    """Raw AP at element offset relative to `ap`'s origin with [step,count] pairs.
    First pattern entry is the partition-dim pair."""
    return bass.AP(tensor=ap.tensor, offset=ap.offset + off_elems, ap=pattern)


def _flat(ap3):
    """[P, a, b] view -> [P, a*b]."""
    return ap3.rearrange("p a b -> p (a b)")


def build_nc(variant="v1", repeat=1):
    nc = bacc.Bacc(num_swdge_queues=NUM_SWDGE_QUEUES)
    if variant == "v3":
        return build_v3(nc)
    rows = nc.dram_tensor("rows", [ROWS_PAD // 3, 255], F32, kind="ExternalInput")
    ch4 = nc.dram_tensor("ch4", [128, 396], F32, kind="ExternalInput")
    gt = nc.dram_tensor("gt", [NP, 4], F32, kind="ExternalInput")
    lbl = nc.dram_tensor("lbl", [NP, 1], F32, kind="ExternalInput")
    cc = nc.dram_tensor("cc", [1, 18], F32, kind="ExternalInput")
    anc0 = nc.dram_tensor("anc0", [3, 80, 80, 4], F32, kind="ExternalInput")
    anc1 = nc.dram_tensor("anc1", [3, 40, 40, 4], F32, kind="ExternalInput")
    anc2 = nc.dram_tensor("anc2", [3, 20, 20, 4], F32, kind="ExternalInput")
    out = nc.dram_tensor("out", [1, 18], F32, kind="ExternalOutput")
    dbg_g = nc.dram_tensor("dbg_g", [NP, 9 * 85], F32, kind="ExternalOutput") if DEBUG_G else None
    dbg_i = nc.dram_tensor("dbg_i", [NP, 3], I32, kind="ExternalOutput") if DEBUG_G else None

    with tile.TileContext(nc) as tc:
        for _rep in range(repeat):
            with tc.tile_pool(name=f"sing{_rep}", bufs=1) as sg, \
                 tc.tile_pool(name=f"dchunk{_rep}", bufs=3) as dpool, \
                 tc.tile_pool(name=f"dram{_rep}", bufs=1, space="DRAM") as drp, \
                 tc.tile_pool(name=f"psum{_rep}", bufs=1, space="PSUM") as psp:

                V = nc.vector

                # ---------------- loads ----------------
                ccb = sg.tile([NP, 6, 3], F32)      # const groups x scales
                cc0 = cc[:, :]
                nc.gpsimd.dma_start(out=ccb[:], in_=_rap(cc0, 0, [[0, NP], [3, 6], [1, 3]]))

                gtt = sg.tile([NP, 4], F32)
                nc.sync.dma_start(out=gtt[:], in_=gt[:, :])
                lblc = sg.tile([NP, 1], F32)
                nc.sync.dma_start(out=lblc[:], in_=lbl[:, :])

                ANC = sg.tile([NP, 3, 3, 4], F32)   # (s, a, xyxy) of cell (0,0)
                for s, anc in enumerate((anc0, anc1, anc2)):
                    a0 = anc[:, :, :, :]
                    nc.gpsimd.dma_start(
                        out=ANC[:, s, :, :],
                        in_=_rap(a0, 0, [[0, NP], [HWS[s] * 4, 3], [1, 4]]))

                def cg(g):  # [NP,3] const view, per scale
                    return ccb[:, g, :]

                def b9(col):  # [NP,1] -> [NP,9] free broadcast
                    return col.to_broadcast([NP, 9])

                def b3(col):
                    return col.to_broadcast([NP, 3])

                def r3(ap2d):  # [NP,9] -> [NP,3,3]
                    return ap2d.rearrange("p (s a) -> p s a", a=3)

                def mk9_from_s(src3):
                    """materialize [NP,9] tile broadcasting a per-scale [NP,3] over a"""
                    t = sg.tile([NP, 9], F32, tag=f"mk9_{nc.next_id()}")
                    src = bass.AP(tensor=src3.tensor, offset=src3.offset,
                                  ap=[src3.ap[0], src3.ap[1], [0, 3]])
                    V.tensor_copy(r3(t[:]), src)
                    return t

                # ---------------- dense: sum softplus(ch4) ----------------
                dsum = sg.tile([128, 3], F32)
                V.memset(dsum[:], 0.0)
                if variant == "v2":
                    c4t = sg.tile([128, 396], F32)
                    nc.sync.dma_start(out=c4t[:], in_=ch4[:, :])
                    cb = [0, 300, 375]
                    for s in range(3):
                        w = NCOL[s]
                        e = dpool.tile([128, 300], F32, tag="dexp")
                        nc.scalar.activation(out=e[:, :w], in_=c4t[:, cb[s]:cb[s] + w],
                                             func=AF.Exp)
                        sp = dpool.tile([128, 300], F32, tag="dsp")
                        nc.scalar.activation(out=sp[:, :w], in_=e[:, :w], func=AF.Ln,
                                             bias=1.0, accum_out=dsum[:, s:s + 1])
                else:
                    r0 = rows[:, :]
                    accs = []
                    for i, (s, c0, w) in enumerate(CHUNKS):
                        t = dpool.tile([128, 75, 85], F32, tag="dch")
                        nc.sync.dma_start(
                            out=t[:, :w, :],
                            in_=_rap(r0, (SBASE[s] + c0) * 85,
                                     [[NCOL[s] * 85, 128], [85, w], [1, 85]]))
                        e = dpool.tile([128, 75], F32, tag="dexp")
                        nc.scalar.activation(out=e[:, :w], in_=_flat(t[:, :w, 4:5]),
                                             func=AF.Exp)
                        sp = dpool.tile([128, 75], F32, tag="dsp")
                        acc = sg.tile([128, 1], F32, tag=f"dacc{i}")
                        nc.scalar.activation(out=sp[:, :w], in_=e[:, :w], func=AF.Ln,
                                             bias=1.0, accum_out=acc[:])
                        accs.append((s, acc))
                    for s, acc in accs:
                        V.tensor_add(dsum[:, s:s + 1], dsum[:, s:s + 1], acc[:])

                # ---------------- sparse: entry geometry ----------------
                x1, y1 = gtt[:, 0:1], gtt[:, 1:2]
                x2, y2 = gtt[:, 2:3], gtt[:, 3:4]
                gcx = sg.tile([NP, 1], F32)
                V.tensor_add(gcx[:], x1, x2)
                V.tensor_scalar_mul(gcx[:], gcx[:], 0.5)
                gcy = sg.tile([NP, 1], F32)
                V.tensor_add(gcy[:], y1, y2)
                V.tensor_scalar_mul(gcy[:], gcy[:], 0.5)

                def floor_clip(src, dst, tagp):
                    # dst[NP,3] = clip(trunc-toward-floor(src * inv_stride), 0, W-1)
                    V.tensor_mul(dst[:], b3(src[:]), cg(0))
                    ti = sg.tile([NP, 3], I32, tag=f"{tagp}_i")
                    V.tensor_copy(ti[:], dst[:])
                    tf = sg.tile([NP, 3], F32, tag=f"{tagp}_f")
                    V.tensor_copy(tf[:], ti[:])
                    adj = sg.tile([NP, 3], F32, tag=f"{tagp}_a")
                    V.tensor_tensor(out=adj[:], in0=tf[:], in1=dst[:], op=OP.is_gt)
                    V.tensor_sub(dst[:], tf[:], adj[:])
                    V.tensor_scalar_max(dst[:], dst[:], 0.0)
                    V.tensor_tensor(out=dst[:], in0=dst[:], in1=cg(2), op=OP.min)

                gx = sg.tile([NP, 3], F32)
                floor_clip(gcx, gx, "fcx")
                gy = sg.tile([NP, 3], F32)
                floor_clip(gcy, gy, "fcy")

                ck = sg.tile([NP, 3], F32)  # cell key per scale
                V.tensor_mul(ck[:], gy[:], cg(3))
                V.tensor_add(ck[:], ck[:], gx[:])

                # entry/partition index columns
                pidx = sg.tile([NP, 1], I32)
                nc.gpsimd.iota(pidx[:], pattern=[[0, 1]], base=0, channel_multiplier=1)
                pcol = sg.tile([NP, 1], F32)
                V.tensor_copy(pcol[:], pidx[:])
                bsel = sg.tile([NP, 1], F32)   # 1.0 for local image 1 (p >= 48)
                V.tensor_single_scalar(out=bsel[:], in_=pcol[:], scalar=47.5, op=OP.is_gt)

                stride9 = mk9_from_s(cg(1))

                # rows are ordered [b, cell, a] per scale; view them as triples
                # of 255 floats so one gather pulls an entry's 3 anchor rows.
                # triple index = base3_s + b*HW_s + cell
                idxf = sg.tile([NP, 3], F32)
                V.tensor_mul(idxf[:], b3(bsel[:]), cg(4))
                V.tensor_add(idxf[:], idxf[:], ck[:])
                V.tensor_add(idxf[:], idxf[:], cg(5))
                idx = sg.tile([NP, 3], I32)
                V.tensor_copy(idx[:], idxf[:])

                # ---------------- gathers ----------------
                # separate destination tiles so the 3 indirect DMAs pipeline
                # (slice-writes into one tile serialize on WAW tracking);
                # repack afterwards with cheap DVE copies.
                G = sg.tile([NP, 9, 85], F32)
                if GATHER_OFF:
                    V.memset(G[:], 0.1)
                else:
                    gks = []
                    for k in range(3):
                        # flat [NP, 255] dest: the SWDGE ucode scales indices by the
                        # dest's innermost contiguous run, which must be 255
                        gk = sg.tile([NP, 255], F32, tag=f"gk{k}")
                        nc.gpsimd.indirect_dma_start(
                            out=gk[:], out_offset=None, in_=rows[:, :],
                            in_offset=bass.IndirectOffsetOnAxis(ap=idx[:, k:k + 1], axis=0))
                        gks.append(gk)
                    for k in range(3):
                        V.tensor_copy(G[:, k * 3:(k + 1) * 3, :],
                                      gks[k][:].rearrange("p (a b) -> p a b", b=85))
                if DEBUG_G and _rep == 0:
                    nc.sync.dma_start(out=dbg_g[:, :], in_=G[:].rearrange("p a b -> p (a b)"))
                    nc.sync.dma_start(out=dbg_i[:, :], in_=idx[:])

                # ---------------- anchor boxes & IoU ----------------
                AW = sg.tile([NP, 9], F32)
                V.tensor_sub(r3(AW[:]), ANC[:, :, :, 2], ANC[:, :, :, 0])
                AH = sg.tile([NP, 9], F32)
                V.tensor_sub(r3(AH[:]), ANC[:, :, :, 3], ANC[:, :, :, 1])
                AWH = sg.tile([NP, 9], F32)
                V.tensor_scalar_mul(AWH[:], AW[:], 0.5)
                AHH = sg.tile([NP, 9], F32)
                V.tensor_scalar_mul(AHH[:], AH[:], 0.5)

                acx = sg.tile([NP, 3], F32)
                V.tensor_scalar_add(acx[:], gx[:], 0.5)
                V.tensor_mul(acx[:], acx[:], cg(1))
                acy = sg.tile([NP, 3], F32)
                V.tensor_scalar_add(acy[:], gy[:], 0.5)
                V.tensor_mul(acy[:], acy[:], cg(1))
                acx9 = mk9_from_s(acx[:])
                acy9 = mk9_from_s(acy[:])

                ax1 = sg.tile([NP, 9], F32)
                V.tensor_sub(ax1[:], acx9[:], AWH[:])
                ax2 = sg.tile([NP, 9], F32)
                V.tensor_add(ax2[:], acx9[:], AWH[:])
                ay1 = sg.tile([NP, 9], F32)
                V.tensor_sub(ay1[:], acy9[:], AHH[:])
                ay2 = sg.tile([NP, 9], F32)
                V.tensor_add(ay2[:], acy9[:], AHH[:])

                ag = sg.tile([NP, 1], F32)
                w2 = sg.tile([NP, 1], F32)
                h2 = sg.tile([NP, 1], F32)
                V.tensor_sub(w2[:], x2, x1)
                V.tensor_sub(h2[:], y2, y1)
                V.tensor_mul(ag[:], w2[:], h2[:])

                aarea = sg.tile([NP, 9], F32)
                V.tensor_mul(aarea[:], AW[:], AH[:])

                # IoU(gt, anchor_box) = inter / (area_gt + area_anchor - inter + eps)
                t1 = sg.tile([NP, 9], F32)
                t2 = sg.tile([NP, 9], F32)
                iw0 = sg.tile([NP, 9], F32)
                V.tensor_tensor(out=t1[:], in0=ax1[:], in1=b9(x1), op=OP.max)
                V.tensor_tensor(out=t2[:], in0=ax2[:], in1=b9(x2), op=OP.min)
                V.tensor_sub(iw0[:], t2[:], t1[:])
                V.tensor_scalar_max(iw0[:], iw0[:], 0.0)
                ih0 = sg.tile([NP, 9], F32)
                V.tensor_tensor(out=t1[:], in0=ay1[:], in1=b9(y1), op=OP.max)
                V.tensor_tensor(out=t2[:], in0=ay2[:], in1=b9(y2), op=OP.min)
                V.tensor_sub(ih0[:], t2[:], t1[:])
                V.tensor_scalar_max(ih0[:], ih0[:], 0.0)
                inter0 = sg.tile([NP, 9], F32)
                V.tensor_mul(inter0[:], iw0[:], ih0[:])
                un0 = sg.tile([NP, 9], F32)
                V.tensor_add(un0[:], b9(ag[:]), aarea[:])
                V.tensor_sub(un0[:], un0[:], inter0[:])
                V.tensor_scalar_add(un0[:], un0[:], EPS)
                V.reciprocal(un0[:], un0[:])
                iou = sg.tile([NP, 9], F32)
                V.tensor_mul(iou[:], inter0[:], un0[:])

                # pos / best-anchor fallback
                pos0 = sg.tile([NP, 9], F32)
                V.tensor_single_scalar(out=pos0[:], in_=iou[:], scalar=0.5, op=OP.is_gt)

                def sa(t, a):  # [NP,3] strided per-anchor view of a [NP,9] tile
                    return bass.AP(tensor=t.tensor, offset=t.offset + a,
                                   ap=[t.ap[0], [3, 3]])

                i0, i1, i2 = sa(iou[:], 0), sa(iou[:], 1), sa(iou[:], 2)
                ge01 = sg.tile([NP, 3], F32)
                V.tensor_tensor(out=ge01[:], in0=i0, in1=i1, op=OP.is_ge)
                ge02 = sg.tile([NP, 3], F32)
                V.tensor_tensor(out=ge02[:], in0=i0, in1=i2, op=OP.is_ge)
                ge12 = sg.tile([NP, 3], F32)
                V.tensor_tensor(out=ge12[:], in0=i1, in1=i2, op=OP.is_ge)
                best = sg.tile([NP, 9], F32)
                bb0, bb1, bb2 = sa(best[:], 0), sa(best[:], 1), sa(best[:], 2)
                V.tensor_mul(bb0, ge01[:], ge02[:])
                t3 = sg.tile([NP, 3], F32)
                V.tensor_scalar(out=t3[:], in0=ge01[:], scalar1=-1.0, scalar2=1.0,
                                op0=OP.mult, op1=OP.add)
                V.tensor_mul(bb1, t3[:], ge12[:])
                V.tensor_scalar(out=t3[:], in0=bb0, scalar1=-1.0, scalar2=1.0,
                                op0=OP.mult, op1=OP.add)
                V.tensor_sub(bb2, t3[:], bb1)

                anyp = sg.tile([NP, 3], F32)
                V.tensor_reduce(out=anyp[:], in_=r3(pos0[:]), axis=AX.X, op=OP.max)
                nanyp = sg.tile([NP, 3], F32)
                V.tensor_scalar(out=nanyp[:], in0=anyp[:], scalar1=-1.0, scalar2=1.0,
                                op0=OP.mult, op1=OP.add)
                anyp9 = mk9_from_s(anyp[:])
                nanyp9 = mk9_from_s(nanyp[:])
                posf = sg.tile([NP, 9], F32)
                V.tensor_mul(posf[:], pos0[:], anyp9[:])
                tb = sg.tile([NP, 9], F32)
                V.tensor_mul(tb[:], best[:], nanyp9[:])
                V.tensor_add(posf[:], posf[:], tb[:])

                # ---------------- decode + CIoU ----------------
                sig = sg.tile([NP, 9, 4], F32)
                nc.scalar.activation(out=sig[:], in_=G[:, :, 0:4], func=AF.Exp, scale=-1.0)
                V.tensor_scalar_add(_flat(sig[:]), _flat(sig[:]), 1.0)
                V.reciprocal(_flat(sig[:]), _flat(sig[:]))

                def sigc(i):  # [NP,9] view of sigmoid column i
                    return _flat(sig[:, :, i:i + 1])

                pcx = sg.tile([NP, 9], F32)
                V.tensor_scalar(out=pcx[:], in0=sigc(0), scalar1=2.0, scalar2=-1.0,
                                op0=OP.mult, op1=OP.add)
                V.tensor_mul(pcx[:], pcx[:], stride9[:])
                V.tensor_add(pcx[:], pcx[:], acx9[:])
                pcy = sg.tile([NP, 9], F32)
                V.tensor_scalar(out=pcy[:], in0=sigc(1), scalar1=2.0, scalar2=-1.0,
                                op0=OP.mult, op1=OP.add)
                V.tensor_mul(pcy[:], pcy[:], stride9[:])
                V.tensor_add(pcy[:], pcy[:], acy9[:])

                pw = sg.tile([NP, 9], F32)
                V.tensor_scalar_mul(pw[:], sigc(2), 2.0)
                V.tensor_mul(pw[:], pw[:], pw[:])
                V.tensor_mul(pw[:], pw[:], AW[:])
                ph = sg.tile([NP, 9], F32)
                V.tensor_scalar_mul(ph[:], sigc(3), 2.0)
                V.tensor_mul(ph[:], ph[:], ph[:])
                V.tensor_mul(ph[:], ph[:], AH[:])

                pwh = sg.tile([NP, 9], F32)
                V.tensor_scalar_mul(pwh[:], pw[:], 0.5)
                px1 = sg.tile([NP, 9], F32)
                V.tensor_sub(px1[:], pcx[:], pwh[:])
                px2 = sg.tile([NP, 9], F32)
                V.tensor_add(px2[:], pcx[:], pwh[:])
                V.tensor_scalar_mul(pwh[:], ph[:], 0.5)
                py1 = sg.tile([NP, 9], F32)
                V.tensor_sub(py1[:], pcy[:], pwh[:])
                py2 = sg.tile([NP, 9], F32)
                V.tensor_add(py2[:], pcy[:], pwh[:])

                w1 = sg.tile([NP, 9], F32)
                V.tensor_sub(w1[:], px2[:], px1[:])
                h1 = sg.tile([NP, 9], F32)
                V.tensor_sub(h1[:], py2[:], py1[:])
                w2h2 = sg.tile([NP, 1], F32)
                V.tensor_mul(w2h2[:], w2[:], h2[:])

                # overlap with gt
                V.tensor_tensor(out=t1[:], in0=px1[:], in1=b9(x1), op=OP.max)
                V.tensor_tensor(out=t2[:], in0=px2[:], in1=b9(x2), op=OP.min)
                iw = sg.tile([NP, 9], F32)
                V.tensor_sub(iw[:], t2[:], t1[:])
                V.tensor_scalar_max(iw[:], iw[:], 0.0)
                V.tensor_tensor(out=t1[:], in0=py1[:], in1=b9(y1), op=OP.max)
                V.tensor_tensor(out=t2[:], in0=py2[:], in1=b9(y2), op=OP.min)
                ih = sg.tile([NP, 9], F32)
                V.tensor_sub(ih[:], t2[:], t1[:])
                V.tensor_scalar_max(ih[:], ih[:], 0.0)
                inter = sg.tile([NP, 9], F32)
                V.tensor_mul(inter[:], iw[:], ih[:])
                un = sg.tile([NP, 9], F32)
                V.tensor_mul(un[:], w1[:], h1[:])
                V.tensor_add(un[:], un[:], b9(w2h2[:]))
                V.tensor_sub(un[:], un[:], inter[:])
                V.tensor_scalar_add(un[:], un[:], EPS)
                V.reciprocal(un[:], un[:])
                iou2 = sg.tile([NP, 9], F32)
                V.tensor_mul(iou2[:], inter[:], un[:])

                cw = sg.tile([NP, 9], F32)
                V.tensor_tensor(out=t1[:], in0=px2[:], in1=b9(x2), op=OP.max)
                V.tensor_tensor(out=t2[:], in0=px1[:], in1=b9(x1), op=OP.min)
                V.tensor_sub(cw[:], t1[:], t2[:])
                chh = sg.tile([NP, 9], F32)
                V.tensor_tensor(out=t1[:], in0=py2[:], in1=b9(y2), op=OP.max)
                V.tensor_tensor(out=t2[:], in0=py1[:], in1=b9(y1), op=OP.min)
                V.tensor_sub(chh[:], t1[:], t2[:])
                c2t = sg.tile([NP, 9], F32)
                V.tensor_mul(c2t[:], cw[:], cw[:])
                V.tensor_mul(t1[:], chh[:], chh[:])
                V.tensor_add(c2t[:], c2t[:], t1[:])
                V.tensor_scalar_add(c2t[:], c2t[:], EPS)

                gx12 = sg.tile([NP, 1], F32)
                V.tensor_add(gx12[:], x1, x2)
                gy12 = sg.tile([NP, 1], F32)
                V.tensor_add(gy12[:], y1, y2)
                rho = sg.tile([NP, 9], F32)
                V.tensor_sub(rho[:], b9(gx12[:]), px1[:])
                V.tensor_sub(rho[:], rho[:], px2[:])
                V.tensor_mul(rho[:], rho[:], rho[:])
                rhoy = sg.tile([NP, 9], F32)
                V.tensor_sub(rhoy[:], b9(gy12[:]), py1[:])
                V.tensor_sub(rhoy[:], rhoy[:], py2[:])
                V.tensor_mul(rhoy[:], rhoy[:], rhoy[:])
                V.tensor_add(rho[:], rho[:], rhoy[:])
                V.tensor_scalar_mul(rho[:], rho[:], 0.25)

                # v term: atan(r2) - atan(r1) == atan((r2-r1)/(1+r1*r2)) for r1,r2>0
                r2c = sg.tile([NP, 1], F32)
                V.tensor_scalar_add(r2c[:], h2[:], EPS)
                V.reciprocal(r2c[:], r2c[:])
                V.tensor_mul(r2c[:], r2c[:], w2[:])
                r1t = sg.tile([NP, 9], F32)
                V.tensor_scalar_add(r1t[:], h1[:], EPS)
                V.reciprocal(r1t[:], r1t[:])
                V.tensor_mul(r1t[:], r1t[:], w1[:])
                num = sg.tile([NP, 9], F32)
                V.tensor_sub(num[:], b9(r2c[:]), r1t[:])
                den = sg.tile([NP, 9], F32)
                V.tensor_mul(den[:], b9(r2c[:]), r1t[:])
                V.tensor_scalar_add(den[:], den[:], 1.0)
                V.reciprocal(den[:], den[:])
                uu = sg.tile([NP, 9], F32)
                V.tensor_mul(uu[:], num[:], den[:])

                au = sg.tile([NP, 9], F32)
                V.tensor_scalar_mul(au[:], uu[:], -1.0)
                V.tensor_tensor(out=au[:], in0=au[:], in1=uu[:], op=OP.max)
                rau = sg.tile([NP, 9], F32)
                V.tensor_scalar_max(rau[:], au[:], 1e-30)
                V.reciprocal(rau[:], rau[:])
                zz = sg.tile([NP, 9], F32)
                V.tensor_tensor(out=zz[:], in0=au[:], in1=rau[:], op=OP.min)
                zq = sg.tile([NP, 9], F32)
                V.tensor_mul(zq[:], zz[:], zz[:])
                poly = sg.tile([NP, 9], F32)
                V.memset(poly[:], ATAN_C[-1])
                for coef in ATAN_C[-2::-1]:
                    V.tensor_mul(poly[:], poly[:], zq[:])
                    V.tensor_scalar_add(poly[:], poly[:], coef)
                V.tensor_mul(poly[:], poly[:], zz[:])
                gt1 = sg.tile([NP, 9], F32)
                V.tensor_single_scalar(out=gt1[:], in_=au[:], scalar=1.0, op=OP.is_gt)
                pm = sg.tile([NP, 9], F32)
                V.tensor_scalar(out=pm[:], in0=poly[:], scalar1=-1.0,
                                scalar2=float(np.pi / 2), op0=OP.mult, op1=OP.add)
                V.tensor_sub(pm[:], pm[:], poly[:])
                V.tensor_mul(pm[:], pm[:], gt1[:])
                at = sg.tile([NP, 9], F32)
                V.tensor_add(at[:], poly[:], pm[:])
                sgn = sg.tile([NP, 9], F32)
                V.tensor_single_scalar(out=sgn[:], in_=uu[:], scalar=0.0, op=OP.is_lt)
                V.tensor_scalar(out=sgn[:], in0=sgn[:], scalar1=-2.0, scalar2=1.0,
                                op0=OP.mult, op1=OP.add)
                V.tensor_mul(at[:], at[:], sgn[:])
                vv = sg.tile([NP, 9], F32)
                V.tensor_mul(vv[:], at[:], at[:])
                V.tensor_scalar_mul(vv[:], vv[:], float(4.0 / (np.pi ** 2)))

                alph = sg.tile([NP, 9], F32)
                V.tensor_sub(alph[:], vv[:], iou2[:])
                V.tensor_scalar(out=alph[:], in0=alph[:], scalar1=1.0, scalar2=EPS,
                                op0=OP.add, op1=OP.add)
                V.reciprocal(alph[:], alph[:])
                V.tensor_mul(alph[:], alph[:], vv[:])    # alpha
                V.tensor_mul(alph[:], alph[:], vv[:])    # v * alpha

                ciou = sg.tile([NP, 9], F32)
                V.reciprocal(c2t[:], c2t[:])
                V.tensor_mul(c2t[:], c2t[:], rho[:])     # rho2 / c2
                V.tensor_add(c2t[:], c2t[:], alph[:])
                V.tensor_sub(ciou[:], iou2[:], c2t[:])

                ciout = sg.tile([NP, 9], F32)
                V.tensor_scalar_max(ciout[:], ciou[:], 0.0)
                V.tensor_scalar_min(ciout[:], ciout[:], 1.0)

                # ---------------- transpose round trip ----------------
                pack = sg.tile([NP, 32], F32)
                negones = sg.tile([NP, 1], F32)
                V.memset(negones[:], -1.0)
                V.tensor_copy(pack[:, 0:9], iou[:])
                notpos = sg.tile([NP, 9], I32)
                V.tensor_single_scalar(out=notpos[:], in_=posf[:], scalar=0.5, op=OP.is_lt)
                V.copy_predicated(pack[:, 0:9], notpos[:], negones[:].to_broadcast([NP, 9]))
                if FULL_DEDUP:
                    V.tensor_copy(pack[:, 9:18], ciout[:])
                    V.tensor_copy(pack[:, 18:21], ck[:])
                    V.tensor_copy(pack[:, 21:22], lblc[:])
                    V.tensor_copy(pack[:, 22:23], bsel[:])
                    V.memset(pack[:, 23:32], 0.0)
                    NRB = 23
                    CKR, LBR, BSR = 18, 21, 22
                else:
                    V.tensor_copy(pack[:, 9:12], ck[:])
                    V.tensor_copy(pack[:, 12:13], bsel[:])
                    V.memset(pack[:, 13:32], 0.0)
                    NRB = 13
                    CKR, LBR, BSR = 9, 21, 12

                T = sg.tile([32, NP], F32)
                for blk in range(3):
                    V.transpose(out=T[:, blk * 32:(blk + 1) * 32],
                                in_=pack[blk * 32:(blk + 1) * 32, :])
                dsc = drp.tile([32, NP], F32)
                nc.sync.dma_start(out=dsc[:], in_=T[:])
                RB = sg.tile([NP, NRB, NP], F32, tag="RB")
                d0 = dsc[:, :]
                nc.gpsimd.dma_start(out=RB[:], in_=_rap(d0, 0, [[0, NP], [NP, NRB], [1, NP]]))

                def rbrow(r):  # [NP, NP] view of transposed row r
                    return RB[:, r:r + 1, :].rearrange("p o n -> p (o n)")

                # ---------------- same-cell logic (full 96-wide, batch mask folded) ---
                beq = sg.tile([NP, NP], F32)   # same local image
                V.tensor_scalar(out=beq[:], in0=rbrow(BSR), scalar1=bsel[:, :],
                                scalar2=None, op0=OP.is_equal)
                sm3 = sg.tile([NP, 3, NP], F32)
                for s in range(3):
                    ksl = sm3[:, s:s + 1, :].rearrange("p o n -> p (o n)")
                    V.tensor_scalar(out=ksl, in0=rbrow(CKR + s), scalar1=ck[:, s:s + 1],
                                    scalar2=None, op0=OP.is_equal)
                    V.tensor_mul(ksl, ksl, beq[:])
                same9 = sg.tile([NP, 9, NP], F32)   # broadcast over a
                s0 = sm3[:, :, :]
                sm4 = bass.AP(tensor=s0.tensor, offset=s0.offset,
                              ap=[s0.ap[0], [NP, 3], [0, 3], [1, NP]])
                V.tensor_copy(same9[:].rearrange("p (s a) n -> p s a n", a=3), sm4)

                nots9 = sg.tile([NP, 9, NP], I32)
                V.tensor_single_scalar(out=nots9[:], in_=same9[:], scalar=0.5, op=OP.is_lt)
                negt = sg.tile([NP, 9, NP], F32)
                V.memset(negt[:], -1.0)

                mv = sg.tile([NP, 9, NP], F32)
                V.tensor_copy(mv[:], RB[:, 0:9, :])
                V.copy_predicated(mv[:], nots9[:], negt[:])

                cellmax = sg.tile([NP, 9], F32)
                V.tensor_reduce(out=cellmax[:], in_=mv[:], axis=AX.X, op=OP.max)

                win = sg.tile([NP, 9], F32)
                V.tensor_tensor(out=win[:], in0=iou[:], in1=cellmax[:], op=OP.is_equal)
                V.tensor_mul(win[:], win[:], posf[:])

                if FULL_DEDUP:
                    wmask = sg.tile([NP, 9, NP], F32)
                if FULL_DEDUP:
                    cm = cellmax[:]
                    cmb = bass.AP(tensor=cm.tensor, offset=cm.offset,
                                  ap=[cm.ap[0], [1, 9], [0, NP]])
                    V.tensor_tensor(out=wmask[:], in0=mv[:], in1=cmb, op=OP.is_equal)

                    objt = sg.tile([NP, 9], F32)
                    wct = sg.tile([NP, 9, NP], F32)
                    V.tensor_mul(wct[:], wmask[:], RB[:, 9:18, :])
                    V.tensor_reduce(out=objt[:], in_=wct[:], axis=AX.X, op=OP.max)

                    # ltm[p, n'] = 1 if n' < p  (global entry order)
                    jrow_i = sg.tile([NP, NP], I32)
                    nc.gpsimd.iota(jrow_i[:], pattern=[[1, NP]], base=0, channel_multiplier=0)
                    jrow = sg.tile([NP, NP], F32)
                    V.tensor_copy(jrow[:], jrow_i[:])
                    ltm = sg.tile([NP, NP], F32)
                    V.tensor_scalar(out=ltm[:], in0=jrow[:], scalar1=pcol[:, :], scalar2=None,
                                    op0=OP.is_lt)
                    lt = ltm[:]
                    ltb = bass.AP(tensor=lt.tensor, offset=lt.offset,
                                  ap=[lt.ap[0], [0, 9], [1, NP]])
                    wl = sg.tile([NP, 9, NP], F32)
                    V.tensor_mul(wl[:], wmask[:], ltb)
                    excl = sg.tile([NP, 9], F32)
                    V.tensor_reduce(out=excl[:], in_=wl[:], axis=AX.X, op=OP.max)
                    rep = sg.tile([NP, 9], F32)
                    V.tensor_scalar(out=rep[:], in0=excl[:], scalar1=-1.0, scalar2=1.0,
                                    op0=OP.mult, op1=OP.add)
                    V.tensor_mul(rep[:], rep[:], win[:])

                    leq = sg.tile([NP, NP], F32)
                    V.tensor_scalar(out=leq[:], in0=rbrow(21), scalar1=lblc[:, :],
                                    scalar2=None, op0=OP.is_equal)
                    lq = leq[:]
                    lqb = bass.AP(tensor=lq.tensor, offset=lq.offset,
                                  ap=[lq.ap[0], [0, 9], [1, NP]])
                    V.tensor_mul(wl[:], wl[:], lqb)
                    exclc = sg.tile([NP, 9], F32)
                    V.tensor_reduce(out=exclc[:], in_=wl[:], axis=AX.X, op=OP.max)
                    repcl = sg.tile([NP, 9], F32)
                    V.tensor_scalar(out=repcl[:], in0=exclc[:], scalar1=-1.0, scalar2=1.0,
                                    op0=OP.mult, op1=OP.add)
                    V.tensor_mul(repcl[:], repcl[:], win[:])
                else:
                    # no bitwise-IoU ties => exactly one winner per cell:
                    # rep == repcl == win, obj target == own clipped ciou
                    rep = win
                    repcl = win
                    objt = ciout

                # ---------------- per-entry loss pieces ----------------
                p4v = _flat(G[:, :, 4:5])
                e4 = sg.tile([NP, 9], F32)
                nc.scalar.activation(out=e4[:], in_=p4v, func=AF.Exp)
                sp4 = sg.tile([NP, 9], F32)
                nc.scalar.activation(out=sp4[:], in_=e4[:], func=AF.Ln, bias=1.0)

                EC = sg.tile([NP, 9, 80], F32)
                nc.scalar.activation(out=EC[:], in_=G[:, :, 5:85], func=AF.Exp)
                nc.scalar.activation(out=EC[:], in_=EC[:], func=AF.Ln, bias=1.0)
                rs9 = sg.tile([NP, 9], F32)
                V.tensor_reduce(out=rs9[:], in_=EC[:], axis=AX.X, op=OP.add)

                ohi = sg.tile([NP, 80], I32)
                nc.gpsimd.iota(ohi[:], pattern=[[1, 80]], base=0, channel_multiplier=0)
                oh = sg.tile([NP, 80], F32)
                V.tensor_copy(oh[:], ohi[:])
                V.tensor_scalar(out=oh[:], in0=oh[:], scalar1=lblc[:, :], scalar2=None,
                                op0=OP.is_equal)
                og = oh[:]
                ohb = bass.AP(tensor=og.tensor, offset=og.offset,
                              ap=[og.ap[0], [0, 9], [1, 80]])
                PL = sg.tile([NP, 9, 80], F32)
                V.tensor_mul(PL[:], G[:, :, 5:85], ohb)
                pl9 = sg.tile([NP, 9], F32)
                V.tensor_reduce(out=pl9[:], in_=PL[:], axis=AX.X, op=OP.add)

                # ---------------- accumulate to 18 outputs ----------------
                pack18 = sg.tile([128, 18], F32)
                V.memset(pack18[96:128, 0:15], 0.0)

                def col3(q):  # strided [NP,3] view of pack18 cols {q, q+5, q+10}
                    sl = pack18[0:96, :]
                    return bass.AP(tensor=sl.tensor, offset=sl.offset + q,
                                   ap=[sl.ap[0], [5, 3]])

                def red3(src_ap, q):
                    V.tensor_reduce(out=col3(q), in_=r3(src_ap), axis=AX.X, op=OP.add)

                tacc = sg.tile([NP, 9], F32)
                V.tensor_scalar(out=tacc[:], in0=ciou[:], scalar1=-1.0, scalar2=1.0,
                                op0=OP.mult, op1=OP.add)
                V.tensor_mul(tacc[:], tacc[:], win[:])
                red3(tacc[:], 0)

                t4 = sg.tile([NP, 9], F32)
                V.tensor_copy(t4[:], p4v)
                V.tensor_mul(t4[:], t4[:], objt[:])
                V.tensor_sub(t4[:], sp4[:], t4[:])
                V.tensor_mul(t4[:], t4[:], rep[:])
                red3(t4[:], 1)

                V.tensor_mul(tacc[:], rep[:], rs9[:])
                t5 = sg.tile([NP, 9], F32)
                V.tensor_mul(t5[:], repcl[:], pl9[:])
                V.tensor_sub(tacc[:], tacc[:], t5[:])
                red3(tacc[:], 2)

                V.tensor_mul(tacc[:], rep[:], sp4[:])
                red3(tacc[:], 3)

                red3(rep[:], 4)

                for s in range(3):
                    V.tensor_copy(pack18[:, 15 + s:16 + s], dsum[:, s:s + 1])

                ones = sg.tile([128, 1], F32)
                V.memset(ones[:], 1.0)
                red_ps = psp.tile([128, 18], F32)
                nc.tensor.matmul(red_ps[:1], ones[:], pack18[:], start=True, stop=True)
                osb = sg.tile([1, 18], F32)
                V.tensor_copy(osb[:], red_ps[:1])
                nc.gpsimd.dma_start(out=out[:, :], in_=osb[:])

    nc.finalize()
    return nc


# ------------------------------------------------------------------
# v3: restructured kernel.
#   - one [NP, MCOLS] "meta" DMA carries gt boxes + per-partition layout
#     constants (no gpsimd broadcast DMAs, no iota ops)
#   - one multi-index indirect gather pulls all 9 anchor rows per entry
#   - same-cell dedup via DVE transpose + PE outer-product broadcasts
#     (PSUM), replacing the DRAM round trip
#   - x/y-paired [NP,18] vector ops + fused scalar_tensor_tensor ops
#   - arctan on the Scalar engine (sigmoid_and_others table set),
#     everything else exp/ln (natural_log_exp set): exactly 2 table loads
# ------------------------------------------------------------------
MCOLS = 144
ATAN_ACT = True
GATHER1 = True
POOL_PL = True
POOL_MV = False


def _bcast(apx, pattern, off=0):
    """Free-dim broadcast/strided view of a slice AP (keeps partition pair)."""
    return bass.AP(tensor=apx.tensor, offset=apx.offset + off,
                   ap=[apx.ap[0]] + pattern)


def build_v3(nc):
    rows = nc.dram_tensor("rows", [ROWS_PAD // 3, 256], F32, kind="ExternalInput")
    ch4 = nc.dram_tensor("ch4", [128, 396], F32, kind="ExternalInput")
    metaa = nc.dram_tensor("metaa", [NP, 176], F32, kind="ExternalInput")
    metab = nc.dram_tensor("metab", [NP, 120], F32, kind="ExternalInput")
    sel = nc.dram_tensor("sel", [32, 12 * NP], F32, kind="ExternalInput")
    out45 = nc.dram_tensor("out45", [NP, 45], F32, kind="ExternalOutput")
    outd = nc.dram_tensor("outd", [128, 3], F32, kind="ExternalOutput")
    CIOU_C = float(4.0 / (np.pi ** 2))
    HB = NP // 2

    with tile.TileContext(nc) as tc:
        with tc.tile_pool(name="sg", bufs=1) as sg, \
             tc.tile_pool(name="psum", bufs=1, space="PSUM") as psp:
            V = nc.vector
            A_ = nc.scalar
            GP = nc.gpsimd

            # ---------------- input DMAs (front-critical meta_a first) ----
            Ma = sg.tile([NP, 176], F32)
            nc.sync.dma_start(out=Ma[:], in_=metaa[:, :])
            Mb = sg.tile([NP, 120], F32)
            nc.sync.dma_start(out=Mb[:], in_=metab[:, :])
            c4t = sg.tile([128, 396], F32)
            nc.sync.dma_start(out=c4t[:], in_=ch4[:, :])
            SEL = sg.tile([32, 12, NP], F32)
            nc.sync.dma_start(out=SEL[:], in_=sel[:, :])

            # meta views
            gt1 = Ma[:, 0:2]
            gt2 = Ma[:, 2:4]
            keyoff = Ma[:, 4:7]
            lblc = Ma[:, 7:8]
            inv2s6 = Ma[:, 8:14]
            W3 = Ma[:, 14:17]
            stride6 = Ma[:, 17:23]
            SEL16 = Ma[:, 24:152]           # [96,128]: k%16 one-hot
            M24 = Ma[:, 152:176]           # [96,24]: k//16 == s%8 mask
            anc1p = _bcast(Mb[:, 0:36], [[4, 9], [1, 2]])       # (x1,y1) pairs
            anc2p = _bcast(Mb[:, 0:36], [[4, 9], [1, 2]], 2)    # (x2,y2) pairs
            iota80 = Mb[:, 40:120]
            gt1_b = _bcast(gt1, [[0, 9], [1, 2]])
            gt2_b = _bcast(gt2, [[0, 9], [1, 2]])

            # ---------------- dense: sum softplus(ch4) per scale ----------
            dacc = sg.tile([128, 3], F32)
            e4d = sg.tile([128, 396], F32)
            A_.activation(out=e4d[:], in_=c4t[:], func=AF.Exp)
            cb = [0, 300, 375]
            for s in range(3):
                w = NCOL[s]
                A_.activation(out=e4d[:, cb[s]:cb[s] + w],
                              in_=e4d[:, cb[s]:cb[s] + w], func=AF.Ln,
                              bias=1.0, accum_out=dacc[:, s:s + 1])
            nc.sync.dma_start(out=outd[:, :], in_=dacc[:])

            # ---------------- front: cell indices + one dma_gather --------
            packi = sg.tile([NP, 32], F32)   # transposed-idx pack (cols 0:3)
            with tc.high_priority():
                V.memset(packi[:, 3:32], 0.0)
                sxy = sg.tile([NP, 2], F32)
                V.tensor_tensor(out=sxy[:], in0=gt1, in1=gt2, op=OP.add)
                gxy0 = sg.tile([NP, 6], F32)    # (s, xy) scaled
                V.tensor_tensor(out=gxy0[:],
                                in0=_bcast(sxy[:], [[0, 3], [1, 2]]),
                                in1=inv2s6, op=OP.mult)
                # floor via round(x - 0.5): f32->i32 convert rounds to nearest
                V.tensor_scalar_add(gxy0[:], gxy0[:], -0.5)
                gxyi = sg.tile([NP, 6], I32)
                V.tensor_copy(gxyi[:], gxy0[:])
                gxy6 = sg.tile([NP, 6], F32)
                V.tensor_copy(gxy6[:], gxyi[:])
                gx3 = _bcast(gxy6[:], [[2, 3]])
                gy3 = _bcast(gxy6[:], [[2, 3]], 1)
                ckt = sg.tile([NP, 3], F32)
                V.tensor_tensor(out=ckt[:], in0=gy3, in1=W3, op=OP.mult)
                V.tensor_tensor(out=ckt[:], in0=ckt[:], in1=gx3, op=OP.add)
                idxf = packi[:, 0:3]
                V.tensor_tensor(out=idxf, in0=ckt[:], in1=keyoff, op=OP.add)
                # replicated [16,24] int16 index block via PE one-hot matmul
                idxmov = sg.tile([NP, 24], F32)
                V.tensor_tensor(out=idxmov[:],
                                in0=_bcast(idxf, [[1, 3], [0, 8]]),
                                in1=M24, op=OP.mult)
                psI = psp.tile([128, 24], F32)
                nc.tensor.matmul(psI[:], SEL16, idxmov[:], start=True, stop=True)
                idx16 = sg.tile([128, 24], mybir.dt.int16)
                V.tensor_copy(idx16[:], psI[:])
                G = sg.tile([128, 3, 256], F32)
                nc.gpsimd.dma_gather(
                    out_ap=G[:], in_ap=rows[:, :], idxs_ap=idx16[:],
                    num_idxs=384, num_idxs_reg=384, elem_size=256)
                # idx transpose for the keq broadcasts
                TI = sg.tile([32, NP], F32)
                for blk in range(3):
                    V.transpose(out=TI[:, blk * 32:(blk + 1) * 32],
                                in_=packi[blk * 32:(blk + 1) * 32, :])

            psB = [psp.tile([NP, 3, NP], F32, name=f"psB{i}") for i in range(4)]
            for j in range(3):
                nc.tensor.matmul(psB[3][:, j, :], SEL[:, j, :], TI[:],
                                 start=True, stop=True)

            def gv(off, pattern):
                g96 = G[0:NP, :, :]
                return _rap(g96, off, [g96.ap[0]] + pattern)

            # ---------------- anchor boxes + anchor IoU (meta-only) -------
            w2h2v = sg.tile([NP, 2], F32)
            V.tensor_tensor(out=w2h2v[:], in0=gt2, in1=gt1, op=OP.subtract)
            ag = sg.tile([NP, 1], F32)
            V.tensor_tensor(out=ag[:], in0=w2h2v[:, 0:1], in1=w2h2v[:, 1:2],
                            op=OP.mult)
            r2c = sg.tile([NP, 1], F32)     # w2/h2 (gt aspect, pred-free)
            V.reciprocal(r2c[:], w2h2v[:, 1:2])
            V.tensor_tensor(out=r2c[:], in0=r2c[:], in1=w2h2v[:, 0:1], op=OP.mult)
            sxy_b = _bcast(sxy[:], [[0, 9], [1, 2]])
            awh = sg.tile([NP, 18], F32)    # (aw, ah) pairs per slot
            V.tensor_tensor(out=awh[:], in0=anc2p, in1=anc1p, op=OP.subtract)
            awhh = sg.tile([NP, 18], F32)   # (aw/2, ah/2)
            V.tensor_scalar_mul(awhh[:], awh[:], 0.5)
            awh2 = sg.tile([NP, 18], F32)   # (2aw, 2ah)
            V.tensor_scalar_mul(awh2[:], awh[:], 2.0)
            aarea = sg.tile([NP, 9], F32)
            V.tensor_tensor(out=aarea[:], in0=_bcast(awh[:], [[2, 9]]),
                            in1=_bcast(awh[:], [[2, 9]], 1), op=OP.mult)

            acxy = sg.tile([NP, 6], F32)    # cell centers (s, xy)
            V.scalar_tensor_tensor(out=acxy[:], in0=gxy6[:], scalar=0.5,
                                   in1=stride6, op0=OP.add, op1=OP.mult)
            acm = sg.tile([NP, 6], F32)     # acxy - stride
            V.scalar_tensor_tensor(out=acm[:], in0=gxy6[:], scalar=-0.5,
                                   in1=stride6, op0=OP.add, op1=OP.mult)
            acxy_b = _bcast(acxy[:], [[2, 3], [0, 3], [1, 2]])
            acm18 = sg.tile([NP, 18], F32)
